# revision 8
# baseline (speedup 1.0000x reference)
"""Trainium2 Bass kernel for nn_CustomAttention_45689862094989.

Reference math (B=2, S=4096, D=1024):
    q = h @ Wq.T + bq ; k = h @ Wk.T + bk
    out = softmax(q @ k.T) @ v                       -> [B, S, 1, D]

Key algebraic reduction: softmax over k is invariant to per-row (q) constant
shifts, so with M = Wq.T @ Wk and vvec = Wk.T @ bq:
    scores ~ (h M) h.T + (h vvec) 1.T        (bk and all q-side bias terms cancel)
Defining GT[d, q] = sum_d'' M[d'', d] HT[d'', q] + vvec[d] (HT = h.T), score
tiles are plain matmuls  scores[q, k] = sum_d GT[d, q] * HT[d, k]  with both
operands already in [contract-on-partition] layout -- no weight transposes and
no K-projection at all.

Sharding: core c -> batch c//4, q-block (c%4)*1024. Host rotates H and V rows
per core so each core's own q-block rows come first; softmax/AV over k are
order-invariant, so the SPMD program is identical across cores. K-dim processed
in 4 resident phases of 1024 with online softmax merging across phases.

All matmuls run as float32r (fp32 storage, ~13 mantissa bits in the PE,
1 cycle/row). fp32 everywhere else.
"""

import numpy as np

import concourse.mybir as mybir
import concourse.tile as tile
from concourse import bacc
from concourse.bass_utils import run_bass_kernel_spmd
from concourse.masks import make_identity

B, S, D = 2, 4096, 1024
P = 128
NCORES = 8
QB = 1024                 # q rows per core

F32 = mybir.dt.float32
F32R = mybir.dt.float32r
AX = mybir.AxisListType.X
OP = mybir.AluOpType
ACTF = mybir.ActivationFunctionType


def build_program(s=S, nph=4, qb=QB):
    kp = s // nph             # k rows per phase
    kc = kp // P              # 128-chunks of k per phase
    sw = min(512, kp)         # score tile width
    nt = kp // sw             # score tiles per phase
    nqt = qb // P             # q tiles per core
    dc = D // P               # contraction chunks
    net = D // 512            # AV output tiles

    nc = bacc.Bacc("TRN2", target_bir_lowering=False, debug=False)
    h = nc.dram_tensor("h", [s, D], F32, kind="ExternalInput")
    v = nc.dram_tensor("v", [s, D], F32, kind="ExternalInput")
    wq = nc.dram_tensor("wq", [D, D], F32, kind="ExternalInput")
    wk = nc.dram_tensor("wk", [D, D], F32, kind="ExternalInput")
    bq = nc.dram_tensor("bq", [D], F32, kind="ExternalInput")
    o = nc.dram_tensor("o", [qb, D], F32, kind="ExternalOutput")

    with tile.TileContext(nc) as tc:
        with (
            tc.tile_pool(name="sb", bufs=1) as sb,
            tc.tile_pool(name="ps", bufs=1, space="PSUM") as ps,
        ):
            # ---- constants ----------------------------------------------
            ident = sb.tile([P, P], F32, tag="ident")
            make_identity(nc, ident[:])
            identr = sb.tile([P, P], F32R, tag="identr")
            nc.vector.tensor_copy(identr[:], ident[:])

            bq_sb = sb.tile([P, dc, 2], F32, tag="bqc")
            nc.vector.memset(bq_sb[:], 0.0)
            nc.sync.dma_start(bq_sb[:, :, 0:1],
                              bq.ap().rearrange("(c p) -> p c", p=P))
            bqr = sb.tile([P, dc, 2], F32R, tag="bqr")
            nc.vector.tensor_copy(bqr[:], bq_sb[:])

            # ---- weights: load + round to f32r --------------------------
            # wqr/wkr chunks share slots with later htp/vpr tiles (same tag).
            wqr, wkr = [], []
            for c in range(dc):
                for (w_dram, dst, tg) in ((wq, wqr, "htp"), (wk, wkr, "vpr")):
                    ld = sb.tile([P, D], F32, tag="ld", bufs=6)
                    nc.sync.dma_start(ld[:], w_dram.ap()[c * P:(c + 1) * P, :])
                    wr = sb.tile([P, D], F32R, tag=tg, bufs=dc + 2)
                    nc.scalar.copy(wr[:], ld[:])
                    dst.append(wr)

            # ---- M = Wq.T @ Wk  and vvec = Wk.T @ bq --------------------
            m_sb = sb.tile([P, dc, D], F32R, tag="big")
            for r in range(dc):
                for n in range(D // 512):
                    pm = ps.tile([P, 512], F32, tag="psA", bufs=2)
                    for c in range(dc):
                        nc.tensor.matmul(
                            pm[:], wqr[c][:, r * P:(r + 1) * P],
                            wkr[c][:, n * 512:(n + 1) * 512],
                            start=(c == 0), stop=(c == dc - 1),
                        )
                    nc.scalar.copy(m_sb[:, r, n * 512:(n + 1) * 512], pm[:])
            v_sb = sb.tile([P, dc], F32, tag="vvec")
            for r in range(dc):
                pv = ps.tile([P, 2], F32, tag="psA", bufs=2)
                for c in range(dc):
                    nc.tensor.matmul(
                        pv[:], wkr[c][:, r * P:(r + 1) * P], bqr[:, c, :],
                        start=(c == 0), stop=(c == dc - 1),
                    )
                nc.vector.tensor_copy(v_sb[:, r:r + 1], pv[:, 0:1])

            # ---- persistent state --------------------------------------
            gt_sb = sb.tile([P, dc, qb], F32R, tag="gt")
            out_sb = sb.tile([P, nqt, D], F32, tag="big")  # aliases m_sb slot
            stats = sb.tile([P, nqt, 2], F32, tag="stats")  # m_run, s_run

            for ph in range(nph):
                # ---- load h rows, transpose to HT; load+round v ---------
                htp = [sb.tile([P, kp], F32R, tag="htp", bufs=dc + 2,
                               name=f"htp{ph}_{i}") for i in range(dc)]
                vpr = []
                for sc in range(kc):
                    r0 = ph * kp + sc * P
                    hp = sb.tile([P, D], F32, tag="ld", bufs=6)
                    nc.sync.dma_start(hp[:], h.ap()[r0:r0 + P, :])
                    for d in range(dc):
                        pt = ps.tile([P, P], F32, tag="psA", bufs=2)
                        nc.tensor.transpose(pt[:], hp[:, d * P:(d + 1) * P],
                                            ident[:])
                        nc.vector.tensor_copy(
                            htp[d][:, sc * P:(sc + 1) * P], pt[:])
                    vt = sb.tile([P, D], F32, tag="ld", bufs=6)
                    nc.sync.dma_start(vt[:], v.ap()[r0:r0 + P, :])
                    vr = sb.tile([P, D], F32R, tag="vpr", bufs=dc + 2)
                    nc.scalar.copy(vr[:], vt[:])
                    vpr.append(vr)

                # ---- GT (phase 0 holds this core's own q rows) ----------
                if ph == 0:
                    assert kp >= qb, "phase 0 must cover the q block"
                    gw = min(512, qb)
                    for r in range(dc):
                        for n in range(qb // gw):
                            pg = ps.tile([P, gw], F32, tag="psA", bufs=2)
                            for c in range(dc):
                                nc.tensor.matmul(
                                    pg[:], m_sb[:, c, r * P:(r + 1) * P],
                                    htp[c][:, n * gw:(n + 1) * gw],
                                    start=(c == 0), stop=(c == dc - 1),
                                )
                            # GT = psum + vvec[d] (ACT Identity bias folds it)
                            nc.scalar.activation(
                                gt_sb[:, r, n * gw:(n + 1) * gw], pg[:],
                                ACTF.Identity, bias=v_sb[:, r:r + 1], scale=1.0,
                            )

                # ---- q tiles -------------------------------------------
                for qt in range(nqt):
                    sc8 = sb.tile([P, 8], F32, tag="sc8", bufs=3)
                    m_run = stats[:, qt, 0:1]
                    s_run = stats[:, qt, 1:2]

                    pss = []
                    for n in range(nt):
                        p_ = ps.tile([P, sw], F32, tag="pss", bufs=2)
                        for c in range(dc):
                            nc.tensor.matmul(
                                p_[:], gt_sb[:, c, qt * P:(qt + 1) * P],
                                htp[c][:, n * sw:(n + 1) * sw],
                                start=(c == 0), stop=(c == dc - 1),
                            )
                        nc.vector.reduce_max(sc8[:, n:n + 1], p_[:], axis=AX)
                        pss.append(p_)

                    if nt == 1:
                        nc.vector.tensor_copy(sc8[:, 2:3], sc8[:, 0:1])
                    else:
                        nc.vector.tensor_tensor(
                            sc8[:, 2:3], sc8[:, 0:1], sc8[:, 1:2], op=OP.max)
                    if ph == 0:
                        # m_run = m_phase
                        nc.vector.tensor_copy(m_run, sc8[:, 2:3])
                    else:
                        nc.vector.tensor_tensor(
                            sc8[:, 3:4], m_run, sc8[:, 2:3], op=OP.max)
                        nc.vector.tensor_tensor(
                            sc8[:, 4:5], m_run, sc8[:, 3:4], op=OP.subtract)
                        nc.scalar.activation(
                            sc8[:, 5:6], sc8[:, 4:5], ACTF.Exp)  # alpha
                        nc.vector.tensor_copy(m_run, sc8[:, 3:4])
                    nc.vector.tensor_scalar_mul(sc8[:, 6:7], m_run, -1.0)

                    ep = sb.tile([P, kp], F32R, tag="ep", bufs=3)
                    for n in range(nt):
                        nc.scalar.activation(
                            ep[:, n * sw:(n + 1) * sw], pss[n][:], ACTF.Exp,
                            bias=sc8[:, 6:7], scale=1.0,
                            accum_out=sc8[:, n:n + 1],
                        )
                    if nt == 1:
                        nc.vector.tensor_copy(sc8[:, 7:8], sc8[:, 0:1])
                    else:
                        nc.vector.tensor_tensor(
                            sc8[:, 7:8], sc8[:, 0:1], sc8[:, 1:2], op=OP.add)
                    if ph == 0:
                        nc.vector.tensor_copy(s_run, sc8[:, 7:8])
                    else:
                        # s_run = s_run * alpha + s_phase
                        nc.vector.scalar_tensor_tensor(
                            s_run, s_run, sc8[:, 5:6], sc8[:, 7:8],
                            op0=OP.mult, op1=OP.add,
                        )

                    # ---- attnT transposes + AV --------------------------
                    pav = [ps.tile([P, 512], F32, tag="pav", bufs=2,
                                   name=f"pav{ph}_{qt}_{i}") for i in range(net)]
                    for c in range(kc):
                        pt2 = ps.tile([P, P], F32R, tag="pst2", bufs=2)
                        nc.tensor.transpose(
                            pt2[:], ep[:, c * P:(c + 1) * P], identr[:])
                        at = sb.tile([P, P], F32R, tag="at", bufs=4)
                        nc.scalar.copy(at[:], pt2[:])
                        for et in range(net):
                            nc.tensor.matmul(
                                pav[et][:], at[:],
                                vpr[c][:, et * 512:(et + 1) * 512],
                                start=(c == 0), stop=(c == kc - 1),
                            )
                    for et in range(net):
                        dst = out_sb[:, qt, et * 512:(et + 1) * 512]
                        if ph == 0:
                            nc.vector.tensor_copy(dst, pav[et][:])
                        else:
                            # out = out * alpha + pav
                            nc.vector.scalar_tensor_tensor(
                                dst, dst, sc8[:, 5:6], pav[et][:],
                                op0=OP.mult, op1=OP.add,
                            )

            # ---- finalize: out /= s_run, store -------------------------
            for qt in range(nqt):
                fin = sb.tile([P, 1], F32, tag="fin", bufs=2)
                nc.vector.reciprocal(fin[:], stats[:, qt, 1:2])
                nc.vector.tensor_scalar_mul(
                    out_sb[:, qt, :], out_sb[:, qt, :], fin[:])
                nc.sync.dma_start(
                    o.ap()[qt * P:(qt + 1) * P, :], out_sb[:, qt, :])
    nc.compile()
    return nc


_PROGRAM = None


def _get_program():
    global _PROGRAM
    if _PROGRAM is None:
        _PROGRAM = build_program()
    return _PROGRAM


def kernel(hidden_states, value_states, Wq, bq, Wk, bk):
    """Full-input entry point. Shards across 8 NeuronCores internally."""
    hidden_states = np.ascontiguousarray(np.asarray(hidden_states, dtype=np.float32))
    value_states = np.ascontiguousarray(np.asarray(value_states, dtype=np.float32))
    Wq = np.ascontiguousarray(np.asarray(Wq, dtype=np.float32))
    Wk = np.ascontiguousarray(np.asarray(Wk, dtype=np.float32))
    bq = np.ascontiguousarray(np.asarray(bq, dtype=np.float32))

    nc = _get_program()
    in_maps = []
    for c in range(NCORES):
        b, qb = c // (NCORES // B), c % (NCORES // B)
        r0 = qb * QB
        # rotate rows so this core's q-block comes first (k-order invariant)
        hrot = np.concatenate(
            [hidden_states[b, r0:], hidden_states[b, :r0]], axis=0)
        vrot = np.concatenate(
            [value_states[b, r0:], value_states[b, :r0]], axis=0)
        in_maps.append({"h": hrot, "v": vrot, "wq": Wq, "wk": Wk, "bq": bq})

    res = run_bass_kernel_spmd(nc, in_maps, core_ids=list(range(NCORES)))

    out = np.empty((B, S, 1, D), dtype=np.float32)
    for c in range(NCORES):
        b, qb = c // (NCORES // B), c % (NCORES // B)
        out[b, qb * QB:(qb + 1) * QB, 0, :] = res.results[c]["o"]
    return out


# revision 9
# speedup vs baseline: 1.3041x; 1.3041x over previous
"""Trainium2 Bass kernel for nn_CustomAttention_45689862094989.

Reference math (B=2, S=4096, D=1024):
    q = h @ Wq.T + bq ; k = h @ Wk.T + bk
    out = softmax(q @ k.T) @ v                       -> [B, S, 1, D]

Key algebraic reduction: softmax over k is invariant to per-row (q) constant
shifts, so with M = Wq.T @ Wk and vvec = Wk.T @ bq:
    scores ~ (h M) h.T + (h vvec) 1.T        (bk and all q-side bias terms cancel)
Defining GT[d, q] = sum_d'' M[d'', d] HT[d'', q] + vvec[d] (HT = h.T), score
tiles are plain matmuls  scores[q, k] = sum_d GT[d, q] * HT[d, k]  with both
operands already in [contract-on-partition] layout -- no weight transposes and
no K-projection at all.

Sharding: core c -> batch c//4, q-block (c%4)*1024. Host rotates H and V rows
per core so each core's own q-block rows come first; softmax/AV over k are
order-invariant, so the SPMD program is identical across cores. K-dim processed
in 4 resident phases of 1024 with online softmax merging across phases.

All matmuls run as float32r (fp32 storage, 11 mantissa bits in the PE,
1 cycle/row warm). fp32 everywhere else. Transposes are 4-batched into single
PSUM banks so each PSUM->SBUF copy moves [128,512]; the q-tile loop is
software-pipelined so scores(qt+1) runs on the PE while softmax(qt) runs on
ACT/DVE, keeping the PE HAM-warm.
"""

import numpy as np

import concourse.mybir as mybir
import concourse.tile as tile
from concourse import bacc
from concourse.bass_utils import run_bass_kernel_spmd
from concourse.masks import make_identity

B, S, D = 2, 4096, 1024
P = 128
NCORES = 8
QB = 1024                 # q rows per core

F32 = mybir.dt.float32
F32R = mybir.dt.float32r
AX = mybir.AxisListType.X
OP = mybir.AluOpType
ACTF = mybir.ActivationFunctionType


def build_program(s=S, nph=4, qb=QB):
    kp = s // nph             # k rows per phase
    kc = kp // P              # 128-chunks of k per phase
    sw = min(512, kp)         # score tile width
    nt = kp // sw             # score tiles per phase
    nqt = qb // P             # q tiles per core
    dc = D // P               # contraction chunks
    net = D // 512            # AV output tiles
    tb = min(4, kc)           # transposes batched per psum bank

    nc = bacc.Bacc("TRN2", target_bir_lowering=False, debug=False)
    h = nc.dram_tensor("h", [s, D], F32, kind="ExternalInput")
    v = nc.dram_tensor("v", [s, D], F32, kind="ExternalInput")
    wq = nc.dram_tensor("wq", [D, D], F32, kind="ExternalInput")
    wk = nc.dram_tensor("wk", [D, D], F32, kind="ExternalInput")
    bq = nc.dram_tensor("bq", [D], F32, kind="ExternalInput")
    o = nc.dram_tensor("o", [qb, D], F32, kind="ExternalOutput")

    with tile.TileContext(nc) as tc:
        with (
            tc.tile_pool(name="sb", bufs=1) as sb,
            tc.tile_pool(name="ps", bufs=1, space="PSUM") as ps,
        ):
            # ---- constants + HAM warmup --------------------------------
            ident = sb.tile([P, P], F32, tag="ident")
            make_identity(nc, ident[:])
            identr = sb.tile([P, P], F32R, tag="identr")
            nc.vector.tensor_copy(identr[:], ident[:])
            # fp32 dummy matmuls warm the PE clock while weight DMAs run
            for i in range(10):
                pw = ps.tile([P, P], F32, tag="pst2", bufs=2, name=f"warm{i}")
                nc.tensor.matmul(pw[:], ident[:], ident[:], start=True,
                                 stop=True)

            bq_sb = sb.tile([P, dc, 2], F32, tag="bqc")
            nc.vector.memset(bq_sb[:], 0.0)
            nc.sync.dma_start(bq_sb[:, :, 0:1],
                              bq.ap().rearrange("(c p) -> p c", p=P))
            bqr = sb.tile([P, dc, 2], F32R, tag="bqr")
            nc.vector.tensor_copy(bqr[:], bq_sb[:])

            # ---- weights: load + round to f32r --------------------------
            # wqr/wkr chunks share slots with later htp/vpr tiles (same tag).
            wqr, wkr = [], []
            for c in range(dc):
                for (w_dram, dst, tg) in ((wq, wqr, "htp"), (wk, wkr, "vpr")):
                    ld = sb.tile([P, D], F32, tag="ld", bufs=6)
                    nc.sync.dma_start(ld[:], w_dram.ap()[c * P:(c + 1) * P, :])
                    wr = sb.tile([P, D], F32R, tag=tg, bufs=dc + 2)
                    nc.scalar.copy(wr[:], ld[:])
                    dst.append(wr)

            # ---- M = Wq.T @ Wk  and vvec = Wk.T @ bq --------------------
            m_sb = sb.tile([P, dc, D], F32R, tag="big")
            for r in range(dc):
                for n in range(D // 512):
                    pm = ps.tile([P, 512], F32, tag="pss", bufs=3)
                    for c in range(dc):
                        nc.tensor.matmul(
                            pm[:], wqr[c][:, r * P:(r + 1) * P],
                            wkr[c][:, n * 512:(n + 1) * 512],
                            start=(c == 0), stop=(c == dc - 1),
                        )
                    nc.scalar.copy(m_sb[:, r, n * 512:(n + 1) * 512], pm[:])
            v_sb = sb.tile([P, dc], F32, tag="vvec")
            for r in range(dc):
                pv = ps.tile([P, 2], F32, tag="pst2", bufs=2)
                for c in range(dc):
                    nc.tensor.matmul(
                        pv[:], wkr[c][:, r * P:(r + 1) * P], bqr[:, c, :],
                        start=(c == 0), stop=(c == dc - 1),
                    )
                nc.vector.tensor_copy(v_sb[:, r:r + 1], pv[:, 0:1])

            # ---- persistent state --------------------------------------
            gt_sb = sb.tile([P, dc, qb], F32R, tag="gt")
            out_sb = sb.tile([P, nqt, D], F32, tag="big")  # aliases m_sb slot
            stats = sb.tile([P, nqt, 2], F32, tag="stats")  # m_run, s_run

            for ph in range(nph):
                # ---- load h rows; 4-batched transposes to HT; load v ----
                htp = [sb.tile([P, kp], F32R, tag="htp", bufs=dc + 2,
                               name=f"htp{ph}_{i}") for i in range(dc)]
                vpr = []
                for g in range(kc // tb):
                    hps = []
                    for j in range(tb):
                        scn = g * tb + j
                        r0 = ph * kp + scn * P
                        hp = sb.tile([P, D], F32, tag="ld", bufs=6)
                        nc.sync.dma_start(hp[:], h.ap()[r0:r0 + P, :])
                        hps.append(hp)
                        vt = sb.tile([P, D], F32, tag="ld", bufs=6)
                        nc.sync.dma_start(vt[:], v.ap()[r0:r0 + P, :])
                        vr = sb.tile([P, D], F32R, tag="vpr", bufs=dc + 2)
                        nc.scalar.copy(vr[:], vt[:])
                        vpr.append(vr)
                    for d in range(dc):
                        ptb = ps.tile([P, tb * P], F32, tag="pst2", bufs=2)
                        for j in range(tb):
                            nc.tensor.transpose(
                                ptb[:, j * P:(j + 1) * P],
                                hps[j][:, d * P:(d + 1) * P], ident[:])
                        nc.vector.tensor_copy(
                            htp[d][:, g * tb * P:(g + 1) * tb * P], ptb[:])

                # ---- GT (phase 0 holds this core's own q rows) ----------
                if ph == 0:
                    assert kp >= qb, "phase 0 must cover the q block"
                    gw = min(512, qb)
                    for r in range(dc):
                        for n in range(qb // gw):
                            pg = ps.tile([P, gw], F32, tag="pss", bufs=3)
                            for c in range(dc):
                                nc.tensor.matmul(
                                    pg[:], m_sb[:, c, r * P:(r + 1) * P],
                                    htp[c][:, n * gw:(n + 1) * gw],
                                    start=(c == 0), stop=(c == dc - 1),
                                )
                            # GT = psum + vvec[d] (ACT Identity bias folds it)
                            nc.scalar.activation(
                                gt_sb[:, r, n * gw:(n + 1) * gw], pg[:],
                                ACTF.Identity, bias=v_sb[:, r:r + 1], scale=1.0,
                            )

                # ---- q tiles: software-pipelined ------------------------
                # stage A(qt): scores matmuls ; stage B(qt): stats+exp ;
                # stage C(qt): attnT transposes + AV + out update.
                # Emission: A0 B0 A1 B1 C0 A2 B2 C1 ... so the PE always has
                # scores(qt+1) queued while ACT/DVE run softmax(qt).
                ep_tiles, ps_tiles, scr = {}, {}, {}

                def stage_a(qt, ph=ph, htp=htp):
                    pss = []
                    for n in range(nt):
                        p_ = ps.tile([P, sw], F32, tag="pss", bufs=3,
                                     name=f"pss{ph}_{qt}_{n}")
                        for c in range(dc):
                            nc.tensor.matmul(
                                p_[:], gt_sb[:, c, qt * P:(qt + 1) * P],
                                htp[c][:, n * sw:(n + 1) * sw],
                                start=(c == 0), stop=(c == dc - 1),
                            )
                        pss.append(p_)
                    ps_tiles[qt] = pss

                def stage_b(qt, ph=ph):
                    pss = ps_tiles[qt]
                    sc8 = sb.tile([P, 8], F32, tag="sc8", bufs=3,
                                  name=f"sc8_{ph}_{qt}")
                    scr[qt] = sc8
                    m_run = stats[:, qt, 0:1]
                    s_run = stats[:, qt, 1:2]
                    for n in range(nt):
                        nc.vector.reduce_max(sc8[:, n:n + 1], pss[n][:],
                                             axis=AX)
                    if nt == 1:
                        nc.vector.tensor_copy(sc8[:, 2:3], sc8[:, 0:1])
                    else:
                        nc.vector.tensor_tensor(
                            sc8[:, 2:3], sc8[:, 0:1], sc8[:, 1:2], op=OP.max)
                    if ph == 0:
                        nc.vector.tensor_copy(m_run, sc8[:, 2:3])
                    else:
                        nc.vector.tensor_tensor(
                            sc8[:, 3:4], m_run, sc8[:, 2:3], op=OP.max)
                        nc.vector.tensor_tensor(
                            sc8[:, 4:5], m_run, sc8[:, 3:4], op=OP.subtract)
                        nc.scalar.activation(
                            sc8[:, 5:6], sc8[:, 4:5], ACTF.Exp)  # alpha
                        nc.vector.tensor_copy(m_run, sc8[:, 3:4])
                    nc.vector.tensor_scalar_mul(sc8[:, 6:7], m_run, -1.0)

                    ep = sb.tile([P, kp], F32R, tag="ep", bufs=3,
                                 name=f"ep{ph}_{qt}")
                    ep_tiles[qt] = ep
                    for n in range(nt):
                        nc.scalar.activation(
                            ep[:, n * sw:(n + 1) * sw], pss[n][:], ACTF.Exp,
                            bias=sc8[:, 6:7], scale=1.0,
                            accum_out=sc8[:, n:n + 1],
                        )
                    if nt == 1:
                        nc.vector.tensor_copy(sc8[:, 7:8], sc8[:, 0:1])
                    else:
                        nc.vector.tensor_tensor(
                            sc8[:, 7:8], sc8[:, 0:1], sc8[:, 1:2], op=OP.add)
                    if ph == 0:
                        nc.vector.tensor_copy(s_run, sc8[:, 7:8])
                    else:
                        nc.vector.scalar_tensor_tensor(
                            s_run, s_run, sc8[:, 5:6], sc8[:, 7:8],
                            op0=OP.mult, op1=OP.add,
                        )

                def stage_c(qt, ph=ph, vpr=vpr):
                    ep, sc8 = ep_tiles.pop(qt), scr.pop(qt)
                    ps_tiles.pop(qt)
                    pav = [ps.tile([P, 512], F32, tag="pav", bufs=3,
                                   name=f"pav{ph}_{qt}_{i}")
                           for i in range(net)]
                    for g in range(kc // tb):
                        ptb = ps.tile([P, tb * P], F32R, tag="pst2", bufs=2,
                                      name=f"ptb{ph}_{qt}_{g}")
                        for j in range(tb):
                            nc.tensor.transpose(
                                ptb[:, j * P:(j + 1) * P],
                                ep[:, (g * tb + j) * P:(g * tb + j + 1) * P],
                                identr[:])
                        at = sb.tile([P, tb * P], F32R, tag="at", bufs=3,
                                     name=f"at{ph}_{qt}_{g}")
                        nc.scalar.copy(at[:], ptb[:])
                        for j in range(tb):
                            c = g * tb + j
                            for et in range(net):
                                nc.tensor.matmul(
                                    pav[et][:], at[:, j * P:(j + 1) * P],
                                    vpr[c][:, et * 512:(et + 1) * 512],
                                    start=(c == 0), stop=(c == kc - 1),
                                )
                    for et in range(net):
                        dst = out_sb[:, qt, et * 512:(et + 1) * 512]
                        if ph == 0:
                            nc.vector.tensor_copy(dst, pav[et][:])
                        else:
                            nc.vector.scalar_tensor_tensor(
                                dst, dst, sc8[:, 5:6], pav[et][:],
                                op0=OP.mult, op1=OP.add,
                            )

                stage_a(0)
                stage_b(0)
                for qt in range(nqt):
                    if qt + 1 < nqt:
                        stage_a(qt + 1)
                        stage_b(qt + 1)
                    stage_c(qt)

            # ---- finalize: out /= s_run, store -------------------------
            for qt in range(nqt):
                fin = sb.tile([P, 1], F32, tag="fin", bufs=2)
                nc.vector.reciprocal(fin[:], stats[:, qt, 1:2])
                nc.vector.tensor_scalar_mul(
                    out_sb[:, qt, :], out_sb[:, qt, :], fin[:])
                nc.sync.dma_start(
                    o.ap()[qt * P:(qt + 1) * P, :], out_sb[:, qt, :])
    nc.compile()
    return nc


_PROGRAM = None


def _get_program():
    global _PROGRAM
    if _PROGRAM is None:
        _PROGRAM = build_program()
    return _PROGRAM


def kernel(hidden_states, value_states, Wq, bq, Wk, bk):
    """Full-input entry point. Shards across 8 NeuronCores internally."""
    hidden_states = np.ascontiguousarray(np.asarray(hidden_states, dtype=np.float32))
    value_states = np.ascontiguousarray(np.asarray(value_states, dtype=np.float32))
    Wq = np.ascontiguousarray(np.asarray(Wq, dtype=np.float32))
    Wk = np.ascontiguousarray(np.asarray(Wk, dtype=np.float32))
    bq = np.ascontiguousarray(np.asarray(bq, dtype=np.float32))

    nc = _get_program()
    in_maps = []
    for c in range(NCORES):
        b, qb = c // (NCORES // B), c % (NCORES // B)
        r0 = qb * QB
        # rotate rows so this core's q-block comes first (k-order invariant)
        hrot = np.concatenate(
            [hidden_states[b, r0:], hidden_states[b, :r0]], axis=0)
        vrot = np.concatenate(
            [value_states[b, r0:], value_states[b, :r0]], axis=0)
        in_maps.append({"h": hrot, "v": vrot, "wq": Wq, "wk": Wk, "bq": bq})

    res = run_bass_kernel_spmd(nc, in_maps, core_ids=list(range(NCORES)))

    out = np.empty((B, S, 1, D), dtype=np.float32)
    for c in range(NCORES):
        b, qb = c // (NCORES // B), c % (NCORES // B)
        out[b, qb * QB:(qb + 1) * QB, 0, :] = res.results[c]["o"]
    return out
